# revision 27
# baseline (speedup 1.0000x reference)
"""Trainium2 Bass kernel for the SE(3) deformation model.

reference math (per point):
    w, v, pivot, t = split(network_output, 4)
    theta = |w| + eps ; wn = w/theta ; vn = v/theta
    R = I + sin(theta) K + (1-cos(theta)) K^2          (K = skew(wn))
    p = (theta I + (1-cos) K + (theta-sin) K^2) vn
    out = R (x + pivot) + p - pivot + t - x

Exact rewrite used here (K~ = skew(w) unnormalized, n2 = |w|^2):
    u  = x + pivot
    k1 = sin(theta)/theta ; k2 = (1-cos(theta))/theta^2
    sg = (theta - sin(theta))/theta^3 = (1 - k1)/theta^2
    out = K~ (k1 u + k2 v) + [w (w.h) - n2 h] + v + t,  h = k2 u + sg v
with 1-cos = 2 sin(theta/2)^2 (reuses the range-wrapped sin argument).

v4 design (per-op HW measurements + stall analysis):
  - Software-pipelined 2 stages: iteration i emits the SCALAR CHAIN of
    chunk i+1 (deinterleaves, n2, sqrt, 1/theta, sins, k-coefficients)
    interleaved with the BIG fp16 ops of chunk i, so the long cross-
    engine chain latency hides behind DVE's contiguous 2x-mode work.
  - Engine split: DVE = 15 contiguous fp16 big ops + n2/recip/wrap/
    k-smalls; ACT = w/v deinterleave + activations (ordered for ~2
    act-table loads per chunk); GPSIMD = u=x+pivot (strided), dwh
    plane-sums, final add with interleave-write, input DMA issue.
  - Strided SBUF reads on DVE run exact 1x; strided writes ~1.7x; ACT
    strided copies ~3us; GPSIMD strided two-read adds are 5-12us (the
    v2 a1=v+t was restructured away: out = cross + S + v + t is summed
    as e1=S+cross, e2=e1+v (DVE contiguous), final=e2+t (GPSIMD, t read
    strided once, interleaved write).
  - fp16 everywhere except theta (f32 through recip/range-wrap); sg via
    (1-k1)/th^2 avoids the f32 theta-sin cancellation and 1/th^3
    overflow; inv2 carries 2/th^2 (Square scale=sqrt2) so k2=shq*inv2
    and sg=(1-k1)/2*inv2.
"""

import math

import numpy as np

import concourse.bacc as bacc
import concourse.mybir as mybir
import concourse.tile as tile
from concourse.alu_op_type import AluOpType
from concourse.bass_utils import run_bass_kernel_spmd

AFT = mybir.ActivationFunctionType
F32 = mybir.dt.float32
F16 = mybir.dt.float16

N_TOTAL = 4194304
NCORES = 8
NPC = N_TOTAL // NCORES  # 524288 points per core
P = 128
F_DEF = 512  # points per partition per chunk
SQRT2 = math.sqrt(2.0)


def build_nc(npc: int = NPC, f: int = F_DEF):
    nchunks = npc // (P * f)
    assert nchunks * P * f == npc

    nc = bacc.Bacc("TRN2", target_bir_lowering=False, debug=False)

    pos = nc.dram_tensor("pos", [npc, 3], F32, kind="ExternalInput")
    net = nc.dram_tensor("net", [npc, 12], F32, kind="ExternalInput")
    out = nc.dram_tensor("out", [npc, 3], F32, kind="ExternalOutput")

    pos_r = pos.ap().rearrange("(n p f) c -> n p (f c)", p=P, f=f)
    net_r = net.ap().rearrange("(n p f) c -> n p (f c)", p=P, f=f)
    out_r = out.ap().rearrange("(n p f) c -> n p (f c)", p=P, f=f)

    V = nc.vector
    G = nc.gpsimd
    S = nc.scalar
    mul, add, sub = AluOpType.mult, AluOpType.add, AluOpType.subtract

    with tile.TileContext(nc) as tc:
        with (
            tc.tile_pool(name="io", bufs=3) as io,
            tc.tile_pool(name="vec", bufs=3) as vec,
            tc.tile_pool(name="sc", bufs=3) as sc,
        ):
            def v3(t):
                return t[:, 0 : 3 * f].rearrange("p (c f) -> p c f", c=3)

            def bc3(s_ap):
                return s_ap.unsqueeze(1).to_broadcast((P, 3, f))

            def issue_loads(i):
                x16 = io.tile([P, 3 * f], F16, tag="x", name="x16")
                net16 = io.tile([P, 12 * f], F16, tag="net", name="net16", bufs=3)
                G.dma_start(out=x16[:], in_=pos_r[i])
                G.dma_start(out=net16[:], in_=net_r[i])
                return x16, net16

            def emit_chain_head(x16, net16):
                """ACT deinterleaves + GPSIMD u for one chunk (no DVE ops).
                Returns the chain state; DVE/ACT tail ops are emitted
                interleaved with the previous chunk's bigs."""
                netp = net16[:].rearrange("p (f c) -> p c f", c=12)
                xp = x16[:].rearrange("p (f c) -> p c f", c=3)

                wx_t = vec.tile([P, 5 * f], F16, tag="wx", name="wx_t")
                u3 = vec.tile([P, 3 * f], F16, tag="u3", name="u3")
                v3p = vec.tile([P, 3 * f], F16, tag="v3p", name="v3p")
                sq = vec.tile([P, 3 * f], F16, tag="sq", name="sq")

                def stile(tag, dt=F16):
                    return sc.tile([P, f], dt, tag=tag, name=tag + "_t")

                c = dict(
                    wx=wx_t, u3=u3, v3p=v3p, sq=sq, netp=netp,
                    n2=stile("n2h"), th32=stile("th32", F32),
                    inv32=stile("inv32", F32), thw=stile("thw", F32),
                    s16=stile("s16"), sh16=stile("sh16"), shq=stile("shq"),
                    inv16=stile("inv16"), inv2=stile("inv2"),
                    k1=stile("k1"), k2=stile("k2"), omk1=stile("omk1"),
                    sg=stile("sg16"),
                )
                # critical path only: w-deint -> squares (feeds n2 -> sqrt);
                # extw/v-deint are emitted later (chain_late) off the path
                S.activation(v3(wx_t), netp[:, 0:3, :], AFT.Copy)
                S.activation(v3(sq), v3(wx_t), AFT.Square)
                # u on DVE: strided reads run exact 1x and are contention-
                # immune (GPSIMD took 4.5-14us on this op and thrashed SBUF)
                V.tensor_tensor(v3(u3), xp, netp[:, 6:9, :], add)
                return c

            def chain_n2(c):
                if c is None:
                    return
                V.tensor_tensor(c["n2"][:], c["sq"][:, 0:f], c["sq"][:, f : 2 * f], add)
                V.tensor_tensor(c["n2"][:], c["n2"][:], c["sq"][:, 2 * f : 3 * f], add)
                S.activation(c["th32"][:], c["n2"][:], AFT.Sqrt)

            def chain_mid(c):
                if c is None:
                    return
                V.reciprocal_approx_fast(out=c["inv32"][:], in_=c["th32"][:])
                V.add_range_wrap(c["thw"][:], c["th32"][:], 0.0, math.pi, 2 * math.pi)
                S.activation(c["inv16"][:], c["inv32"][:], AFT.Copy)
                S.activation(c["inv2"][:], c["inv16"][:], AFT.Square, scale=SQRT2)
                S.activation(c["s16"][:], c["thw"][:], AFT.Sin)
                S.activation(c["sh16"][:], c["thw"][:], AFT.Sin, scale=0.5)

            def chain_tail(c):
                if c is None:
                    return
                V.tensor_tensor(c["shq"][:], c["sh16"][:], c["sh16"][:], mul)
                V.tensor_tensor(c["k1"][:], c["s16"][:], c["inv16"][:], mul)
                V.tensor_tensor(c["k2"][:], c["shq"][:], c["inv2"][:], mul)
                V.tensor_scalar(c["omk1"][:], c["k1"][:], -0.5, 0.5, mul, AluOpType.add)
                V.tensor_tensor(c["sg"][:], c["omk1"][:], c["inv2"][:], mul)

            def chain_late(c):
                if c is None:
                    return
                # off-critical-path ACT copies: w-extend + v deinterleave
                wx_t = c["wx"]
                S.activation(wx_t[:, 3 * f : 5 * f], wx_t[:, 0 : 2 * f], AFT.Copy)
                S.activation(v3(c["v3p"]), c["netp"][:, 3:6, :], AFT.Copy)

            def emit_bigs(c, nxt):
                """Contiguous fp16 big ops for chunk(c); the DVE pieces of
                chunk(nxt)'s scalar chain are sandwiched between groups so
                every cross-engine wait has queued big-work ahead of it."""
                wx_t, u3, v3p = c["wx"], c["u3"], c["v3p"]
                gx_t = vec.tile([P, 5 * f], F16, tag="gx", name="gx_t")
                h3 = vec.tile([P, 3 * f], F16, tag="h3", name="h3")
                t1 = vec.tile([P, 3 * f], F16, tag="t1", name="t1")
                pr = vec.tile([P, 3 * f], F16, tag="pr", name="pr")
                cr = vec.tile([P, 3 * f], F16, tag="cr", name="cr")
                dwh = sc.tile([P, f], F16, tag="dwh", name="dwh_t")

                # g = k1 u + k2 v (extended for the cross)
                V.tensor_tensor(v3(gx_t), v3(u3), bc3(c["k1"][:]), mul)
                V.tensor_tensor(v3(t1), v3(v3p), bc3(c["k2"][:]), mul)
                V.tensor_tensor(v3(gx_t), v3(gx_t), v3(t1), add)
                V.tensor_copy(gx_t[:, 3 * f : 5 * f], gx_t[:, 0 : 2 * f])
                # h = k2 u + sg v
                V.tensor_tensor(v3(h3), v3(u3), bc3(c["k2"][:]), mul)
                V.tensor_tensor(v3(t1), v3(v3p), bc3(c["sg"][:]), mul)
                V.tensor_tensor(v3(h3), v3(h3), v3(t1), add)
                chain_n2(nxt)  # next chunk's n2 (ACT square is ready by now)
                # pr = w . h
                V.tensor_tensor(pr[:], wx_t[:, 0 : 3 * f], h3[:], mul)
                V.tensor_tensor(dwh[:], pr[:, 0:f], pr[:, f : 2 * f], add)
                V.tensor_tensor(dwh[:], dwh[:], pr[:, 2 * f : 3 * f], add)
                # cross = w x g
                V.tensor_tensor(cr[:], wx_t[:, f : 4 * f], gx_t[:, 2 * f : 5 * f], mul)
                V.tensor_tensor(t1[:], wx_t[:, 2 * f : 5 * f], gx_t[:, f : 4 * f], mul)
                V.tensor_tensor(cr[:], cr[:], t1[:], sub)
                chain_mid(nxt)  # next chunk's recip/wrap (+ACT tail it feeds)
                # S = w (w.h) - n2 h  (m2 into u3, dead after h)
                V.tensor_tensor(v3(u3), v3(h3), bc3(c["n2"][:]), mul)
                V.tensor_tensor(v3(t1), v3(wx_t), bc3(dwh[:]), mul)
                V.tensor_tensor(t1[:], t1[:], u3[:], sub)
                chain_tail(nxt)  # next chunk's k-coefficients
                chain_late(nxt)  # next chunk's off-path ACT copies
                # a1 = v + t (planar v, one strided t read: exact 1x on DVE)
                V.tensor_tensor(v3(h3), v3(v3p), c["netp"][:, 9:12, :], add)
                V.tensor_tensor(cr[:], cr[:], t1[:], add)   # e1 = S + cross
                return cr, h3

            def emit_final(e1_prev, a1_prev, i_prev):
                # o-planar = e1 + a1 (contiguous DVE add), GPSIMD does the
                # planar->interleaved copy, SWDGE casts f16->f32 on the store.
                opl = io.tile([P, 3 * f], F16, tag="opl", name="opl", bufs=2)
                o16 = io.tile([P, 3 * f], F16, tag="o", name="o16", bufs=2)
                o_pl = o16[:].rearrange("p (f c) -> p c f", c=3)
                V.tensor_tensor(opl[:], e1_prev[:], a1_prev[:], add)
                G.tensor_copy(o_pl, v3(opl))
                G.dma_start(out=out_r[i_prev], in_=o16[:])

            pending = [issue_loads(0)]
            if nchunks > 1:
                pending.append(issue_loads(1))
            chain = emit_chain_head(*pending.pop(0))
            # chunk 0's chain DVE/ACT tail runs standalone (pipeline fill)
            chain_n2(chain)
            chain_mid(chain)
            chain_tail(chain)
            chain_late(chain)
            defer = None
            for i in range(nchunks):
                if i + 2 < nchunks:
                    pending.append(issue_loads(i + 2))
                cur = chain
                nxt = None
                if i + 1 < nchunks:
                    nxt = emit_chain_head(*pending.pop(0))
                e2_t = emit_bigs(cur, nxt)
                chain = nxt
                if defer is not None:
                    emit_final(*defer)
                defer = (*e2_t, i)
            emit_final(*defer)

    nc.compile()
    return nc


_NC_CACHE: dict = {}


def _get_nc():
    if "nc" not in _NC_CACHE:
        _NC_CACHE["nc"] = build_nc()
    return _NC_CACHE["nc"]


def kernel(undeformed_positions: np.ndarray, network_output: np.ndarray) -> np.ndarray:
    pos = np.ascontiguousarray(np.asarray(undeformed_positions, dtype=np.float32))
    net = np.ascontiguousarray(np.asarray(network_output, dtype=np.float32))
    assert pos.shape == (N_TOTAL, 3) and net.shape == (N_TOTAL, 12)

    nc = _get_nc()
    in_maps = [
        {
            "pos": pos[i * NPC : (i + 1) * NPC],
            "net": net[i * NPC : (i + 1) * NPC],
        }
        for i in range(NCORES)
    ]
    res = run_bass_kernel_spmd(nc, in_maps, list(range(NCORES)))
    return np.concatenate([res.results[i]["out"] for i in range(NCORES)], axis=0)
